# revision 25
# baseline (speedup 1.0000x reference)
"""Haar-DWT downsampling + 1x1 conv + BN + ReLU fused Trainium2 kernel.

Math: the Haar DWT (J=1) followed by a 1x1 conv over the 4C subband
channels, inference BN, and ReLU is one linear op + bias + ReLU.  It
folds into a 2x2/stride-2 conv:

    z[o, i, j] = relu( sum_{c,di,dj} Weff[o, c, di, dj] * x[c, 2i+di, 2j+dj]
                       + bias_total[o] )

with Weff/bias_total computed on the host from (W, b, gamma, beta, mean,
var).  On-device this is, per output tile, accumulating matmuls
(contraction K = 64 per (di,dj) combo, duplicated across both partition
halves) + one vector-engine pass (bias + ReLU) reading PSUM.

Sharding: pure data-parallel over batch. B=16 -> 2 images per core on
8 cores.

Perf notes (vs the f32 single-queue baseline at 685 us; now ~99 us):
  - x is cast to bf16 and z returned as bf16 (cast to f32 on host):
    total HBM traffic drops 50.6 -> 25.3 MB/core.  Max rel error goes
    1.4e-4 -> 3.5e-3, well within the 2e-2 gate.
  - x loads are issued as [64 part x 8 KB] descriptor groups: 8 KB
    elements fan out across all 16 SDMA engines (the 32 KB-per-
    partition descriptors of the baseline drained at ~1.6-engine
    concurrency, 52 GB/s; 8 KB elements sustain ~290 GB/s aggregate).
  - DMA traffic is balanced over three queues (sync HWDGE, scalar/ACT
    HWDGE, gpsimd SWDGE) at ~8.4 MB each so loads and stores overlap.
  - bias+ReLU(+bf16 cast) PSUM drain alternates between the DVE
    (tensor_scalar) and ACT (activation) engines per psum tile.
  - matmuls alternate the two K=64 partition halves (PE row groups
    0-1 / 2-3) so consecutive MMs overlap in the array.
Measured: DMA busy ~87 us of ~99 us exec (25.3 MB at ~290 GB/s, the
practical HBM rate here); ~6 us framework preamble; ~5 us drain.
"""

import os
import numpy as np
import ml_dtypes

import concourse.bass as bass
import concourse.bacc as bacc
import concourse.mybir as mybir
from concourse.tile import TileContext
from concourse.bass_utils import run_bass_kernel_spmd

BN_EPS = 1e-5

# Problem shape (hardcoded per harness contract)
B, C, H, W_IMG = 16, 64, 256, 256
COUT = 128
N_CORES = 8
B_LOCAL = B // N_CORES          # 2 images per core
HO, WO = H // 2, W_IMG // 2     # 128 x 128 output image

F32 = mybir.dt.float32
BF16 = mybir.dt.bfloat16

# x-load descriptor geometry: 8 KB per descriptor = 16 rows x 256 cols bf16
XROWS_PER_DESC = int(os.environ.get("KOPT_XROWS", "16"))
XSEG = 32 // XROWS_PER_DESC     # descriptors per 32-row half
XBUFS = int(os.environ.get("KOPT_XBUFS", "6"))

# bisect flags (default = all optimizations on)
OPT_BF16 = os.environ.get("KOPT_BF16", "1") == "1"
OPT_XSPLIT = os.environ.get("KOPT_XSPLIT", "1") == "1"
OPT_ZSCALAR = os.environ.get("KOPT_ZSCALAR", "1") == "1"
OPT_ZBF16 = os.environ.get("KOPT_ZBF16", "1") == "1"


def _fold_weights(W, b, gamma, beta, mean, var):
    """Fold DWT + conv + BN into per-(di,dj) lhsT weights
    [4, 128(K), 128(M=o)] and a per-channel bias [COUT].

    Combo q = di*2 + dj.  K rows 0-63 and 64-127 hold the SAME c-indexed
    weights (duplicated): the kernel packs two K=64 matmuls into the PE
    array (partition halves 0/64), one per h-half of the input tile, and
    lhsT/rhs base partitions must match.
    """
    W = W.astype(np.float64)
    Wll, Wlh, Whl, Whh = W[:, :C], W[:, C:2 * C], W[:, 2 * C:3 * C], W[:, 3 * C:]
    s = (gamma.astype(np.float64) / np.sqrt(var.astype(np.float64) + BN_EPS))
    coef = {
        (0, 0): 0.5 * (Wll + Wlh + Whl + Whh),
        (0, 1): 0.5 * (Wll + Wlh - Whl - Whh),
        (1, 0): 0.5 * (Wll - Wlh + Whl - Whh),
        (1, 1): 0.5 * (Wll - Wlh - Whl + Whh),
    }
    bias_total = (b.astype(np.float64) * s + beta.astype(np.float64)
                  - mean.astype(np.float64) * s)
    lhsT = np.zeros((4, 128, COUT), dtype=np.float64)
    for di in range(2):
        for dj in range(2):
            wq = (coef[(di, dj)] * s[:, None]).T   # [c, o]
            lhsT[di * 2 + dj, 0:C, :] = wq
            lhsT[di * 2 + dj, C:2 * C, :] = wq
    return lhsT.astype(np.float32), bias_total.astype(np.float32)


def build_nc(b_local=B_LOCAL, n_row_blocks=4, run_bacc_compile=True):
    """n_row_blocks: 64-input-row blocks per image (full image = 4)."""
    mm_dt = BF16 if OPT_BF16 else mybir.dt.float32r
    nc = bacc.Bacc(None)
    z_dt = BF16 if OPT_ZBF16 else F32
    x = nc.dram_tensor("x", [b_local, C, H, W_IMG], mm_dt, kind="ExternalInput")
    w_lhsT = nc.dram_tensor("w_lhsT", [4, 128, COUT], mm_dt, kind="ExternalInput")
    bias = nc.dram_tensor("bias", [COUT, 1], F32, kind="ExternalInput")
    z = nc.dram_tensor("z", [b_local, COUT, HO, WO], z_dt, kind="ExternalOutput")

    with TileContext(nc) as tc:
        with (
            tc.tile_pool(name="consts", bufs=1) as cpool,
            tc.tile_pool(name="xin", bufs=XBUFS) as xpool,
            tc.tile_pool(name="psum", bufs=2, space="PSUM") as ppool,
            tc.tile_pool(name="zout", bufs=3) as zpool,
        ):
            # weight/bias loads go on the scalar queue so the first x
            # tile's loads lead the sync queue
            w_sb = []
            for q in range(4):
                wt = cpool.tile([128, COUT], mm_dt, name=f"w{q}_sb")
                nc.scalar.dma_start(out=wt[:], in_=w_lhsT[q])
                w_sb.append(wt)
            bias_sb = cpool.tile([COUT, 1], F32)
            nc.scalar.dma_start(out=bias_sb[:], in_=bias[:])

            zv = z.rearrange("b o (hb i2) j -> b o hb (i2 j)", hb=HO // 16)

            for bi in range(b_local):
                for tb in range(n_row_blocks):
                    # 64 input rows -> 32 output rows; partition =
                    # (hhalf, c): each partition holds 32 contiguous
                    # input rows (16 KB bf16).  Issued as XSEG dmas of
                    # [128 part x 4 KB] so descriptors fan out across
                    # all 16 SDMA engines.
                    xt = xpool.tile([128, 32 * W_IMG], mm_dt)
                    if OPT_XSPLIT:
                        # one dma per (hh, r): plain [64 part, 2048 elem]
                        # slices; descriptor = 8 rows x 256 cols = 4 KB
                        seg = XROWS_PER_DESC * W_IMG   # elems per descriptor
                        for hh in range(2):
                            for r in range(XSEG):
                                r0 = 64 * tb + 32 * hh + XROWS_PER_DESC * r
                                src = x[bi, :, r0:r0 + XROWS_PER_DESC, :]
                                # spread x over the three DMA queues so
                                # each queue carries ~8.4 MB total
                                si = 2 * hh + r if XSEG == 2 else hh
                                if os.environ.get("KOPT_Q3", "1") == "1":
                                    eng = (nc.sync, nc.sync, nc.scalar,
                                           nc.gpsimd)[si % 4]
                                elif OPT_ZSCALAR and si == 3:
                                    eng = (nc.gpsimd
                                           if os.environ.get("KOPT_XGPS") == "1"
                                           else nc.scalar)
                                else:
                                    eng = nc.sync
                                eng.dma_start(
                                    out=xt[64 * hh:64 * (hh + 1),
                                           seg * r:seg * (r + 1)],
                                    in_=src.rearrange("c hl w -> c (hl w)"),
                                )
                    else:
                        src = x[bi, :, 64 * tb:64 * (tb + 1), :].rearrange(
                            "c (hh hl) w -> hh c (hl w)", hh=2
                        )
                        nc.sync.dma_start(out=xt[:], in_=src)
                    # free f = il*512 + di*256 + j*2 + dj  (il<16 per half)
                    xv = xt.rearrange(
                        "p (il di j dj) -> p di dj il j", di=2, j=WO, dj=2
                    )
                    zeng = nc.scalar if OPT_ZSCALAR else nc.sync
                    if OPT_ZBF16:
                        # one zt tile per row block; both psum tiles relu
                        # into it, then one store of [o, hb=2, 4 KB elem]
                        zt4 = zpool.tile([COUT, 4096], z_dt)
                        ztv = zt4.rearrange("o (hh g) -> o hh g", hh=2)
                    for pt in range(2):   # two psum tiles per block
                        ps = ppool.tile([COUT, 2048], F32)
                        # region h*1024 + gg*512 <- output rows
                        # (32tb + 16h + 8pt + 4gg + 0..3).  h innermost:
                        # consecutive matmuls hit different PE row groups
                        # (base partition 0 vs 64) so they overlap in the
                        # array.
                        for q in range(4):
                            di, dj = q // 2, q % 2
                            for gg in range(2):
                                il0 = 8 * pt + 4 * gg
                                for h in range(2):
                                    lw = w_sb[q][h * C:(h + 1) * C, :]
                                    nc.tensor.matmul(
                                        ps[:, h * 1024 + gg * 512:
                                           h * 1024 + gg * 512 + 512],
                                        lhsT=lw,
                                        rhs=xv[h * C:(h + 1) * C, di, dj,
                                               il0:il0 + 4, :],
                                        start=(q == 0),
                                        stop=(q == 3),
                                    )
                        if OPT_ZBF16:
                            # bias + ReLU: max(ps + bias, 0), cast to bf16.
                            # pt=0 on DVE, pt=1 on ACT: splits the PSUM
                            # drain work across both engines.
                            if pt == 0:
                                nc.vector.tensor_scalar(
                                    ztv[:, :, 1024 * pt:1024 * pt + 1024],
                                    ps.rearrange("o (hh f) -> o hh f", hh=2),
                                    bias_sb[:, 0:1], 0.0,
                                    mybir.AluOpType.add, mybir.AluOpType.max,
                                )
                            else:
                                nc.scalar.activation(
                                    ztv[:, :, 1024 * pt:1024 * pt + 1024],
                                    ps.rearrange("o (hh f) -> o hh f", hh=2),
                                    mybir.ActivationFunctionType.Relu,
                                    bias=bias_sb[:, 0:1],
                                )
                        else:
                            zt = zpool.tile([COUT, 2048], F32)
                            nc.vector.tensor_scalar(
                                zt[:], ps[:], bias_sb[:, 0:1], 0.0,
                                mybir.AluOpType.add, mybir.AluOpType.max,
                            )
                            nc.scalar.dma_start(
                                out=zv[bi, :, 2 * tb:2 * tb + 2,
                                       1024 * pt:1024 * pt + 1024],
                                in_=zt.rearrange("o (hh f) -> o hh f", hh=2),
                            ) if OPT_ZSCALAR else nc.sync.dma_start(
                                out=zv[bi, :, 2 * tb:2 * tb + 2,
                                       1024 * pt:1024 * pt + 1024],
                                in_=zt.rearrange("o (hh f) -> o hh f", hh=2),
                            )
                    if OPT_ZBF16:
                        # rows 32tb..32tb+32 as hb groups {2tb, 2tb+1};
                        # alternate scalar/gpsimd queues per block
                        if os.environ.get("KOPT_Q3", "1") == "1":
                            zeng = nc.scalar if tb % 2 == 0 else nc.gpsimd
                        zeng.dma_start(
                            out=zv[bi, :, 2 * tb:2 * tb + 2, :],
                            in_=ztv[:],
                        )
    if run_bacc_compile:
        nc.compile()
    return nc


_NC_CACHE = {}


def _get_nc():
    if "nc" not in _NC_CACHE:
        _NC_CACHE["nc"] = build_nc()
    return _NC_CACHE["nc"]


def kernel(x, W, b, gamma, beta, mean, var, _trace=False):
    in_dt = ml_dtypes.bfloat16 if OPT_BF16 else np.float32
    x_bf = np.asarray(x, dtype=np.float32).astype(in_dt)
    lhsT, bias_total = _fold_weights(
        np.asarray(W), np.asarray(b), np.asarray(gamma),
        np.asarray(beta), np.asarray(mean), np.asarray(var),
    )
    lhsT_bf = lhsT.astype(in_dt)
    bias_col = np.ascontiguousarray(bias_total.reshape(COUT, 1))

    nc = _get_nc()
    in_maps = []
    for core in range(N_CORES):
        xs = np.ascontiguousarray(x_bf[core * B_LOCAL:(core + 1) * B_LOCAL])
        in_maps.append({"x": xs, "w_lhsT": lhsT_bf, "bias": bias_col})

    res = run_bass_kernel_spmd(
        nc, in_maps, list(range(N_CORES)), trace=_trace
    )
    out = np.concatenate([res.results[i]["z"] for i in range(N_CORES)], axis=0)
    out = np.ascontiguousarray(out.astype(np.float32))
    if _trace:
        return out, res
    return out


# revision 26
# speedup vs baseline: 1.0190x; 1.0190x over previous
"""Haar-DWT downsampling + 1x1 conv + BN + ReLU fused Trainium2 kernel.

Math: the Haar DWT (J=1) followed by a 1x1 conv over the 4C subband
channels, inference BN, and ReLU is one linear op + bias + ReLU.  It
folds into a 2x2/stride-2 conv:

    z[o, i, j] = relu( sum_{c,di,dj} Weff[o, c, di, dj] * x[c, 2i+di, 2j+dj]
                       + bias_total[o] )

with Weff/bias_total computed on the host from (W, b, gamma, beta, mean,
var).  On-device this is, per output tile, accumulating matmuls
(contraction K = 64 per (di,dj) combo, duplicated across both partition
halves) + one vector-engine pass (bias + ReLU) reading PSUM.

Sharding: pure data-parallel over batch. B=16 -> 2 images per core on
8 cores.

Perf notes (vs the f32 single-queue baseline at 685 us; now ~99 us):
  - x is cast to bf16 and z returned as bf16 (cast to f32 on host):
    total HBM traffic drops 50.6 -> 25.3 MB/core.  Max rel error goes
    1.4e-4 -> 3.5e-3, well within the 2e-2 gate.
  - x loads are issued as [64 part x 8 KB] descriptor groups: 8 KB
    elements fan out across all 16 SDMA engines (the 32 KB-per-
    partition descriptors of the baseline drained at ~1.6-engine
    concurrency, 52 GB/s; 8 KB elements sustain ~290 GB/s aggregate).
  - DMA traffic is balanced over three queues (sync HWDGE, scalar/ACT
    HWDGE, gpsimd SWDGE) at ~8.4 MB each so loads and stores overlap.
  - bias+ReLU(+bf16 cast) PSUM drain alternates between the DVE
    (tensor_scalar) and ACT (activation) engines per psum tile.
  - matmuls alternate the two K=64 partition halves (PE row groups
    0-1 / 2-3) so consecutive MMs overlap in the array.
Measured: DMA busy ~87 us of ~99 us exec (25.3 MB at ~290 GB/s, the
practical HBM rate here); ~6 us framework preamble; ~5 us drain.
"""

import os
import numpy as np
import ml_dtypes

import concourse.bass as bass
import concourse.bacc as bacc
import concourse.mybir as mybir
from concourse.tile import TileContext
from concourse.bass_utils import run_bass_kernel_spmd

BN_EPS = 1e-5

# Problem shape (hardcoded per harness contract)
B, C, H, W_IMG = 16, 64, 256, 256
COUT = 128
N_CORES = 8
B_LOCAL = B // N_CORES          # 2 images per core
HO, WO = H // 2, W_IMG // 2     # 128 x 128 output image

F32 = mybir.dt.float32
BF16 = mybir.dt.bfloat16

# x-load descriptor geometry: 8 KB per descriptor = 16 rows x 256 cols bf16
XROWS_PER_DESC = int(os.environ.get("KOPT_XROWS", "16"))
XSEG = 32 // XROWS_PER_DESC     # descriptors per 32-row half
XBUFS = int(os.environ.get("KOPT_XBUFS", "6"))

# bisect flags (default = all optimizations on)
OPT_BF16 = os.environ.get("KOPT_BF16", "1") == "1"
OPT_XSPLIT = os.environ.get("KOPT_XSPLIT", "1") == "1"
OPT_ZSCALAR = os.environ.get("KOPT_ZSCALAR", "1") == "1"
OPT_ZBF16 = os.environ.get("KOPT_ZBF16", "1") == "1"


def _fold_weights(W, b, gamma, beta, mean, var):
    """Fold DWT + conv + BN into per-(di,dj) lhsT weights
    [4, 128(K), 128(M=o)] and a per-channel bias [COUT].

    Combo q = di*2 + dj.  K rows 0-63 and 64-127 hold the SAME c-indexed
    weights (duplicated): the kernel packs two K=64 matmuls into the PE
    array (partition halves 0/64), one per h-half of the input tile, and
    lhsT/rhs base partitions must match.
    """
    W = W.astype(np.float64)
    Wll, Wlh, Whl, Whh = W[:, :C], W[:, C:2 * C], W[:, 2 * C:3 * C], W[:, 3 * C:]
    s = (gamma.astype(np.float64) / np.sqrt(var.astype(np.float64) + BN_EPS))
    coef = {
        (0, 0): 0.5 * (Wll + Wlh + Whl + Whh),
        (0, 1): 0.5 * (Wll + Wlh - Whl - Whh),
        (1, 0): 0.5 * (Wll - Wlh + Whl - Whh),
        (1, 1): 0.5 * (Wll - Wlh - Whl + Whh),
    }
    bias_total = (b.astype(np.float64) * s + beta.astype(np.float64)
                  - mean.astype(np.float64) * s)
    lhsT = np.zeros((4, 128, COUT), dtype=np.float64)
    for di in range(2):
        for dj in range(2):
            wq = (coef[(di, dj)] * s[:, None]).T   # [c, o]
            lhsT[di * 2 + dj, 0:C, :] = wq
            lhsT[di * 2 + dj, C:2 * C, :] = wq
    return lhsT.astype(np.float32), bias_total.astype(np.float32)


def build_nc(b_local=B_LOCAL, n_row_blocks=4, run_bacc_compile=True):
    """n_row_blocks: 64-input-row blocks per image (full image = 4)."""
    mm_dt = BF16 if OPT_BF16 else mybir.dt.float32r
    nc = bacc.Bacc(None)
    z_dt = BF16 if OPT_ZBF16 else F32
    x = nc.dram_tensor("x", [b_local, C, H, W_IMG], mm_dt, kind="ExternalInput")
    w_lhsT = nc.dram_tensor("w_lhsT", [4, 128, COUT], mm_dt, kind="ExternalInput")
    bias = nc.dram_tensor("bias", [COUT, 1], F32, kind="ExternalInput")
    z = nc.dram_tensor("z", [b_local, COUT, HO, WO], z_dt, kind="ExternalOutput")

    with TileContext(nc) as tc:
        with (
            tc.tile_pool(name="consts", bufs=1) as cpool,
            tc.tile_pool(name="xin", bufs=XBUFS) as xpool,
            tc.tile_pool(name="psum", bufs=2, space="PSUM") as ppool,
            tc.tile_pool(name="zout",
                         bufs=int(os.environ.get("KOPT_ZBUFS", "3"))) as zpool,
        ):
            # weight/bias loads go on the scalar queue so the first x
            # tile's loads lead the sync queue
            w_sb = []
            for q in range(4):
                wt = cpool.tile([128, COUT], mm_dt, name=f"w{q}_sb")
                nc.scalar.dma_start(out=wt[:], in_=w_lhsT[q])
                w_sb.append(wt)
            bias_sb = cpool.tile([COUT, 1], F32)
            nc.scalar.dma_start(out=bias_sb[:], in_=bias[:])

            zv = z.rearrange("b o (hb i2) j -> b o hb (i2 j)", hb=HO // 16)

            for bi in range(b_local):
                for tb in range(n_row_blocks):
                    # 64 input rows -> 32 output rows; partition =
                    # (hhalf, c): each partition holds 32 contiguous
                    # input rows (16 KB bf16).  Issued as XSEG dmas of
                    # [128 part x 4 KB] so descriptors fan out across
                    # all 16 SDMA engines.
                    xt = xpool.tile([128, 32 * W_IMG], mm_dt)
                    if OPT_XSPLIT:
                        # one dma per (hh, r): plain [64 part, 2048 elem]
                        # slices; descriptor = 8 rows x 256 cols = 4 KB
                        seg = XROWS_PER_DESC * W_IMG   # elems per descriptor
                        for hh in range(2):
                            for r in range(XSEG):
                                r0 = 64 * tb + 32 * hh + XROWS_PER_DESC * r
                                src = x[bi, :, r0:r0 + XROWS_PER_DESC, :]
                                # spread x over the three DMA queues so
                                # each queue carries ~8.4 MB total
                                si = 2 * hh + r if XSEG == 2 else hh
                                if os.environ.get("KOPT_Q3", "1") == "1":
                                    eng = (nc.sync, nc.sync, nc.scalar,
                                           nc.gpsimd)[si % 4]
                                elif OPT_ZSCALAR and si == 3:
                                    eng = (nc.gpsimd
                                           if os.environ.get("KOPT_XGPS") == "1"
                                           else nc.scalar)
                                else:
                                    eng = nc.sync
                                eng.dma_start(
                                    out=xt[64 * hh:64 * (hh + 1),
                                           seg * r:seg * (r + 1)],
                                    in_=src.rearrange("c hl w -> c (hl w)"),
                                )
                    else:
                        src = x[bi, :, 64 * tb:64 * (tb + 1), :].rearrange(
                            "c (hh hl) w -> hh c (hl w)", hh=2
                        )
                        nc.sync.dma_start(out=xt[:], in_=src)
                    # free f = il*512 + di*256 + j*2 + dj  (il<16 per half)
                    xv = xt.rearrange(
                        "p (il di j dj) -> p di dj il j", di=2, j=WO, dj=2
                    )
                    zeng = nc.scalar if OPT_ZSCALAR else nc.sync
                    if OPT_ZBF16:
                        # one zt tile per row block; both psum tiles relu
                        # into it, then one store of [o, hb=2, 4 KB elem]
                        zt4 = zpool.tile([COUT, 4096], z_dt)
                        ztv = zt4.rearrange("o (hh g) -> o hh g", hh=2)
                    for pt in range(2):   # two psum tiles per block
                        ps = ppool.tile([COUT, 2048], F32)
                        # region h*1024 + gg*512 <- output rows
                        # (32tb + 16h + 8pt + 4gg + 0..3).  h innermost:
                        # consecutive matmuls hit different PE row groups
                        # (base partition 0 vs 64) so they overlap in the
                        # array.
                        for q in range(4):
                            di, dj = q // 2, q % 2
                            for gg in range(2):
                                il0 = 8 * pt + 4 * gg
                                for h in range(2):
                                    lw = w_sb[q][h * C:(h + 1) * C, :]
                                    nc.tensor.matmul(
                                        ps[:, h * 1024 + gg * 512:
                                           h * 1024 + gg * 512 + 512],
                                        lhsT=lw,
                                        rhs=xv[h * C:(h + 1) * C, di, dj,
                                               il0:il0 + 4, :],
                                        start=(q == 0),
                                        stop=(q == 3),
                                    )
                        if OPT_ZBF16:
                            # bias + ReLU: max(ps + bias, 0), cast to bf16.
                            # pt=0 on DVE, pt=1 on ACT: splits the PSUM
                            # drain work across both engines.
                            if pt == 0:
                                nc.vector.tensor_scalar(
                                    ztv[:, :, 1024 * pt:1024 * pt + 1024],
                                    ps.rearrange("o (hh f) -> o hh f", hh=2),
                                    bias_sb[:, 0:1], 0.0,
                                    mybir.AluOpType.add, mybir.AluOpType.max,
                                )
                            else:
                                nc.scalar.activation(
                                    ztv[:, :, 1024 * pt:1024 * pt + 1024],
                                    ps.rearrange("o (hh f) -> o hh f", hh=2),
                                    mybir.ActivationFunctionType.Relu,
                                    bias=bias_sb[:, 0:1],
                                )
                        else:
                            zt = zpool.tile([COUT, 2048], F32)
                            nc.vector.tensor_scalar(
                                zt[:], ps[:], bias_sb[:, 0:1], 0.0,
                                mybir.AluOpType.add, mybir.AluOpType.max,
                            )
                            nc.scalar.dma_start(
                                out=zv[bi, :, 2 * tb:2 * tb + 2,
                                       1024 * pt:1024 * pt + 1024],
                                in_=zt.rearrange("o (hh f) -> o hh f", hh=2),
                            ) if OPT_ZSCALAR else nc.sync.dma_start(
                                out=zv[bi, :, 2 * tb:2 * tb + 2,
                                       1024 * pt:1024 * pt + 1024],
                                in_=zt.rearrange("o (hh f) -> o hh f", hh=2),
                            )
                    if OPT_ZBF16:
                        # rows 32tb..32tb+32 as hb groups {2tb, 2tb+1};
                        # alternate scalar/gpsimd queues per block
                        if os.environ.get("KOPT_Q3", "1") == "1":
                            zeng = nc.scalar if tb % 2 == 0 else nc.gpsimd
                        zeng.dma_start(
                            out=zv[bi, :, 2 * tb:2 * tb + 2, :],
                            in_=ztv[:],
                        )
    if run_bacc_compile:
        nc.compile()
    return nc


_NC_CACHE = {}


def _get_nc():
    if "nc" not in _NC_CACHE:
        _NC_CACHE["nc"] = build_nc()
    return _NC_CACHE["nc"]


def kernel(x, W, b, gamma, beta, mean, var, _trace=False):
    in_dt = ml_dtypes.bfloat16 if OPT_BF16 else np.float32
    x_bf = np.asarray(x, dtype=np.float32).astype(in_dt)
    lhsT, bias_total = _fold_weights(
        np.asarray(W), np.asarray(b), np.asarray(gamma),
        np.asarray(beta), np.asarray(mean), np.asarray(var),
    )
    lhsT_bf = lhsT.astype(in_dt)
    bias_col = np.ascontiguousarray(bias_total.reshape(COUT, 1))

    nc = _get_nc()
    in_maps = []
    for core in range(N_CORES):
        xs = np.ascontiguousarray(x_bf[core * B_LOCAL:(core + 1) * B_LOCAL])
        in_maps.append({"x": xs, "w_lhsT": lhsT_bf, "bias": bias_col})

    res = run_bass_kernel_spmd(
        nc, in_maps, list(range(N_CORES)), trace=_trace
    )
    out = np.concatenate([res.results[i]["z"] for i in range(N_CORES)], axis=0)
    out = np.ascontiguousarray(out.astype(np.float32))
    if _trace:
        return out, res
    return out
